# revision 22
# baseline (speedup 1.0000x reference)
"""Gated multi-head self-attention on 8 Trainium2 NeuronCores.

Sharding: batch (B=2) x head-groups (4 groups of 4 heads) -> 8 cores.
Each core computes, for its batch b and its 4 heads:
    partial_out[t, e] = sum_h gate[h] * softmax(Q_h K_h^T / 8) V_h Wo_h
The host sums the 4 head-group partials per batch, adds the constant
term sum_h gate_h*(bo_h + bv_h Wo_h) (bv/bo commute past the softmax
normalization), and stacks the two batches.

Design (ScalarE exp is the floor: 128 x [128,1024] exps ~= 135us/core):
  - fp8e4 DoubleRow matmuls (2 moving cols/cycle, 256-deep contraction)
    for the QKV projections, PV, row-sums, and output projection; the
    scores matmuls stay bf16 (their d=64 contraction can't pair-split
    without restructuring Q/K layouts).
  - exp writes fp8 s-tile-PAIR tiles [128,(head,kt,t)]; PV consumes a
    pair per DoubleRow matmul (contraction 256 = two s-tiles).
  - softmax denominators: fp8 ones-matmuls ride the same exp pairs,
    accumulating both heads' row-sums into psum rows 0/1 of a dedicated
    bank; reciprocal_approx_fast + sel2 matmul broadcasts 1/R.
    This removes the former DVE chain-add stream entirely.
  - host-folded prescales keep fp8 operands in e4m3 normal range:
    Wq,Wk,bq,bk x64 (exp scale 0.125/4096), Wv x32, Wo*gate x64,
    ctx evicted as 32*ctx (sel2 value 1 = 32/32), output eviction
    multiplies by 1/2048 = 1/(64*32).
  - warm-up matmuls + six per-ec-pair interleaved projections ride the
    (bandwidth-bound) hT DMA window before attention starts.
  - attention groups pr-outer; per iteration: scores pair (row-tiled,
    both heads concurrent), one exp, at most one extra-work quantum,
    one spill op from the previous group's tail, this group's lag-4
    PV pair (even iterations) and lag-3 row-sum pair (odd iterations)
    -- so the in-order engine queues never stall the exp stream.
"""

import os
import numpy as np
import ml_dtypes
from contextlib import ExitStack

import concourse.bass as bass
import concourse.tile as tile
from concourse import bacc, mybir
from concourse import bass_utils

E, H, D = 1024, 16, 64
B, T = 2, 2048
NCORES = 8
P = 128
TC = 512          # t-chunk (PSUM bank = 512 fp32)
NTC = T // TC     # 4 t-chunks
NST = T // P      # 16 s-tiles
NEC = E // P      # 8 e-chunks

F32 = mybir.dt.float32
BF16 = mybir.dt.bfloat16
F8 = mybir.dt.float8e4
ADD = mybir.AluOpType.add
MULT = mybir.AluOpType.mult
DR = mybir.MatmulPerfMode.DoubleRow

AQ = 64.0   # Wq/Wk/bq/bk prescale  (exp scale divides by AQ^2)
AV = 32.0   # Wv prescale
CC = 1.0    # ctx_sb = CC * ctx_true  (sel2 value = CC/AV)
EXP_SCALE = 0.125 / (AQ * AQ)


def build_kernel():
    nc = bacc.Bacc("TRN2", target_bir_lowering=False, debug=False,
                   num_devices=NCORES)
    hT = nc.dram_tensor("hT", [NEC, P, T], F8, kind="ExternalInput").ap()
    wq = nc.dram_tensor("wq", [P, 2, 4, 2, P], F8, kind="ExternalInput").ap()
    wk = nc.dram_tensor("wk", [P, 2, 4, 2, P], F8, kind="ExternalInput").ap()
    wv = nc.dram_tensor("wv", [P, 4, 2, 256], F8, kind="ExternalInput").ap()
    wo = nc.dram_tensor("wo", [P, 2, E], BF16, kind="ExternalInput").ap()
    bq = nc.dram_tensor("bq", [P, 2], F32, kind="ExternalInput").ap()
    bk = nc.dram_tensor("bk", [P, 2], F32, kind="ExternalInput").ap()
    on2 = nc.dram_tensor("on2", [P, 2, 2, P], F8, kind="ExternalInput").ap()
    sel2 = nc.dram_tensor("sel2", [2, P], BF16, kind="ExternalInput").ap()
    out = nc.dram_tensor("out", [T, E], BF16, kind="ExternalOutput").ap()

    sim_safe = os.environ.get("BASS_SIM_SAFE", "") != ""

    with tile.TileContext(nc) as tc:
        with ExitStack() as ctx:
            persist = ctx.enter_context(tc.tile_pool(name="persist", bufs=1))
            expool = ctx.enter_context(tc.tile_pool(name="expool", bufs=8))
            work = ctx.enter_context(tc.tile_pool(name="work", bufs=8))
            ps_s = ctx.enter_context(tc.tile_pool(name="ps_s", bufs=2, space="PSUM"))
            ps_ctx = ctx.enter_context(tc.tile_pool(name="ps_ctx", bufs=2, space="PSUM"))
            ps_misc = ctx.enter_context(tc.tile_pool(name="ps_misc", bufs=1, space="PSUM"))
            ps_rs = ctx.enter_context(tc.tile_pool(name="ps_rs", bufs=1, space="PSUM"))

            # ---- persistent SBUF tensors ----
            hT_sb = persist.tile([P, NEC, T], F8, tag="hT")
            wq_sb = persist.tile([P, 2, 4, 2, P], F8, tag="wq")
            wk_sb = persist.tile([P, 2, 4, 2, P], F8, tag="wk")
            wv_sb = persist.tile([P, 4, 2, 256], F8, tag="wv")
            wo_sb = persist.tile([P, 2, E], BF16, tag="wo")
            bq_sb = persist.tile([P, 2], F32, tag="bq")
            bk_sb = persist.tile([P, 2], F32, tag="bk")
            on2_sb = persist.tile([P, 2, 2, P], F8, tag="on2")
            sel2_sb = persist.tile([2, P], BF16, tag="sel2")
            QT_sb = persist.tile([P, 2, T], BF16, tag="QT")
            KT_sb = persist.tile([P, 2, T], BF16, tag="KT")
            # V2: zero-padded dual-head stationary for DoubleRow PV.
            # [s-part, s-tile, pr, head-ktile, d-col]: kt0 cols 0:64 = V_A,
            # kt1 cols 64:128 = V_B, rest stays zero (gpsimd memset).
            V2_sb = persist.tile([P, NST, 2, 2, P], F8, tag="V2")
            ctx_sb = persist.tile([P, 2, T], BF16, tag="ctx")

            with nc.named_scope("load"):
                # PE warm-up matmuls on uninitialized SBUF keep the tensor
                # engine busy during the input DMAs so the HAM clock gate
                # ramps toward 8/8 before the real work starts.  Skipped
                # under BASS_SIM_SAFE (CoreSim rejects uninitialized reads;
                # these ops are timing-only).
                if not sim_safe:
                    for w in range(8):
                        psw = ps_misc.tile([P, TC], F32, tag="ps_misc",
                                           name=f"warm{w}")
                        nc.tensor.matmul(psw[:], KT_sb[0:64, 0, 0:P],
                                         QT_sb[0:64, 0, 0:TC],
                                         start=True, stop=True)
                    for w in range(3):
                        vw = work.tile([2, 64], BF16, tag="vw",
                                       name=f"vwarm{w}")
                        nc.vector.tensor_copy(vw[:], QT_sb[0:2, 0, 0:64])
                # Spread DMA issues across the three idle sequencers.
                nc.sync.dma_start(wk_sb[:], wk)
                nc.scalar.dma_start(wq_sb[:], wq)
                nc.scalar.dma_start(bq_sb[:], bq)
                nc.scalar.dma_start(bk_sb[:], bk)
                for ec in range(NEC):
                    for hp in range(2):
                        eng = (nc.sync, nc.gpsimd, nc.scalar)[(2 * ec + hp) % 3]
                        eng.dma_start(hT_sb[hp * 64:(hp + 1) * 64, ec, :],
                                      hT[ec][hp * 64:(hp + 1) * 64])
                nc.gpsimd.dma_start(wv_sb[:], wv)
                nc.gpsimd.dma_start(wo_sb[:], wo)
                nc.sync.dma_start(on2_sb[:], on2)
                nc.sync.dma_start(sel2_sb[:], sel2)
                nc.gpsimd.memset(V2_sb[:], 0.0)

            def qk_matmuls(w_sb, dst, pr, tch):
                for ecp in range(4):
                    nc.tensor.matmul(
                        dst, w_sb[:, pr, ecp, :, :],
                        hT_sb[:, 2 * ecp:2 * ecp + 2,
                              tch * TC:(tch + 1) * TC],
                        start=(ecp == 0), stop=(ecp == 3),
                        perf_mode=DR, skip_group_check=True)

            def proj_qk(w_sb, b_sb, dst, pr, tch):
                """One [128, TC] chunk of the Q or K projection (+bias)."""
                ps = ps_misc.tile([P, TC], F32, tag="ps_misc",
                                  name=f"pqk_{pr}_{tch}")
                qk_matmuls(w_sb, ps[:], pr, tch)
                nc.vector.tensor_scalar(
                    dst[:, pr, tch * TC:(tch + 1) * TC], ps[:],
                    b_sb[:, pr:pr + 1], None, ADD)

            def proj_v(st):
                """V rows for s-tile st -> zero-padded V2 stationary."""
                ps = ps_misc.tile([P, TC], F32, tag="ps_misc")
                psv = ps[:, :256]
                for ecp in range(4):
                    nc.tensor.matmul(
                        psv, hT_sb[:, 2 * ecp:2 * ecp + 2,
                                   st * P:(st + 1) * P],
                        wv_sb[:, ecp, :, :],
                        start=(ecp == 0), stop=(ecp == 3),
                        perf_mode=DR, skip_group_check=True)
                for pr in range(2):
                    for h in range(2):
                        c = (2 * pr + h) * 64
                        nc.vector.tensor_copy(
                            V2_sb[:, st, pr, h, 64 * h:64 * h + 64],
                            psv[:, c:c + 64])

            def outproj_half(tt, ec2):
                """Half the output projection for t-tile tt -> HBM."""
                pso = ps_misc.tile([P, TC], F32, tag="ps_misc")
                for pr in range(2):
                    nc.tensor.matmul(
                        pso[:], ctx_sb[:, pr, tt * P:(tt + 1) * P],
                        wo_sb[:, pr, ec2 * TC:(ec2 + 1) * TC],
                        start=(pr == 0), stop=(pr == 1),
                        skip_group_check=True)
                o_sb = work.tile([P, TC], BF16, tag="o")
                nc.vector.tensor_copy(o_sb[:], pso[:])
                nc.sync.dma_start(
                    out[tt * P:(tt + 1) * P, ec2 * TC:(ec2 + 1) * TC],
                    o_sb[:])

            def pv(pctx, expair, s, pr):
                """ctx(both heads) += V2[s]^T @ exp[s] in one DoubleRow
                matmul: head k-tiles against the zero-padded stationary."""
                nc.tensor.matmul(
                    pctx[:], V2_sb[:, s, pr, :, :],
                    expair[:, s % 2, :, :],
                    start=(s == 0), stop=(s == NST - 1),
                    perf_mode=DR, skip_group_check=True)

            def rs_pair(prs, expair, p):
                """Row-sums of an s-tile pair: head h lands in prs row h
                (ones column h of the on2 variant), rest accumulates 0."""
                for h in range(2):
                    nc.tensor.matmul(
                        prs[:], on2_sb[:, h, :, :],
                        expair[:, :, h, :],
                        start=(p == 0 and h == 0),
                        stop=(p == NST // 2 - 1 and h == 1),
                        perf_mode=DR, skip_group_check=True)

            with nc.named_scope("qkv"):
                # Six projection chunks (K t0-t3, Q t0-t1) interleaved per
                # e-chunk-pair so the matmuls ride the hT DMA arrivals and
                # the whole K(pr0) set is done by the end of the load.
                psk0 = ps_misc.tile([P, TC], F32, tag="ps_misc", name="psk0")
                psq0 = ps_rs.tile([P, TC], F32, tag="ps_rs", name="psq0")
                pre_a = ps_s.tile([P, 2 * TC], F32, tag="ps_s", name="pre_a")
                pre_b = ps_s.tile([P, 2 * TC], F32, tag="ps_s", name="pre_b")
                jobs = [
                    (wk_sb, psk0[:], 0),            # K t0
                    (wq_sb, psq0[:], 0),            # Q t0
                    (wk_sb, pre_a[:, :TC], 1),      # K t1
                    (wk_sb, pre_a[:, TC:], 2),      # K t2
                    (wk_sb, pre_b[:, :TC], 3),      # K t3
                    (wq_sb, pre_b[:, TC:], 1),      # Q t1
                ]
                for ecp in range(4):
                    for w_sb, dst, t in jobs:
                        nc.tensor.matmul(
                            dst, w_sb[:, 0, ecp, :, :],
                            hT_sb[:, 2 * ecp:2 * ecp + 2,
                                  t * TC:(t + 1) * TC],
                            start=(ecp == 0), stop=(ecp == 3),
                            perf_mode=DR, skip_group_check=True)
                nc.vector.tensor_scalar(
                    KT_sb[:, 0, 0:TC], psk0[:], bk_sb[:, 0:1], None, ADD)
                nc.vector.tensor_scalar(
                    QT_sb[:, 0, 0:TC], psq0[:], bq_sb[:, 0:1], None, ADD)
                nc.vector.tensor_scalar(
                    KT_sb[:, 0, TC:2 * TC], pre_a[:, :TC],
                    bk_sb[:, 0:1], None, ADD)
                nc.vector.tensor_scalar(
                    KT_sb[:, 0, 2 * TC:3 * TC], pre_a[:, TC:],
                    bk_sb[:, 0:1], None, ADD)
                nc.vector.tensor_scalar(
                    KT_sb[:, 0, 3 * TC:4 * TC], pre_b[:, :TC],
                    bk_sb[:, 0:1], None, ADD)
                nc.vector.tensor_scalar(
                    QT_sb[:, 0, TC:2 * TC], pre_b[:, TC:],
                    bq_sb[:, 0:1], None, ADD)

            # Extra-work quanta interleaved one-per-iteration into the
            # attention groups as (min_iteration, fn).
            def QK(wb, bb, dstb, pr, t):
                return lambda: proj_qk(wb, bb, dstb, pr, t)

            extras = {
                0: [(0, QK(wq_sb, bq_sb, QT_sb, 0, 2)),
                    (1, QK(wq_sb, bq_sb, QT_sb, 0, 3))]
                   + [(2 + j, (lambda st=st: proj_v(st)))
                      for j, st in enumerate(range(0, 14))],
                1: [(j, (lambda st=st: proj_v(st)))
                    for j, st in enumerate(range(14, NST))]
                   + [(2 + j, QK(wk_sb, bk_sb, KT_sb, 1, t))
                      for j, t in enumerate(range(4))]
                   + [(6, QK(wq_sb, bq_sb, QT_sb, 1, 0))],
                2: [(0, QK(wq_sb, bq_sb, QT_sb, 1, 1)),
                    (1, QK(wq_sb, bq_sb, QT_sb, 1, 2)),
                    (2, QK(wq_sb, bq_sb, QT_sb, 1, 3))],
                5: [(8 + j, (lambda tt=tt, e=e: outproj_half(tt, e)))
                    for j, (tt, e) in enumerate(
                        (t, x) for t in range(0, 4) for x in range(2))],
                6: [(8 + j, (lambda tt=tt, e=e: outproj_half(tt, e)))
                    for j, (tt, e) in enumerate(
                        (t, x) for t in range(4, 8) for x in range(2))],
                7: [(8 + j, (lambda tt=tt, e=e: outproj_half(tt, e)))
                    for j, (tt, e) in enumerate(
                        (t, x) for t in range(8, 12) for x in range(2))],
            }

            # ---- attention: 8 groups, pr-outer ----
            def tail_pieces(gi, tch, pr, pctx, prs):
                t0 = tch * TC
                state = {}

                def p_rcp():
                    rcp_sb = work.tile([2, TC], F32, tag="rcp",
                                       name=f"rcp_{gi}")
                    nc.vector.reciprocal_approx_fast(rcp_sb[:], prs[0:2, :])
                    rcp_bf = work.tile([2, TC], BF16, tag="rcpb")
                    nc.vector.tensor_copy(rcp_bf[:], rcp_sb[:])
                    state["rcp_bf"] = rcp_bf

                def p_bcast():
                    pR = ps_misc.tile([P, TC], F32, tag="ps_misc",
                                      name=f"pR_{gi}")
                    nc.tensor.matmul(pR[:], sel2_sb[:], state["rcp_bf"][:],
                                     start=True, stop=True,
                                     skip_group_check=True)
                    state["pR"] = pR

                def p_mult():
                    R_sb = work.tile([P, TC], F32, tag="R", name=f"R_{gi}")
                    nc.vector.tensor_copy(R_sb[:], state["pR"][:])
                    h = TC // 2
                    nc.vector.tensor_tensor(
                        ctx_sb[:, pr, t0:t0 + h], pctx[:, :h],
                        R_sb[:, :h], MULT)
                    nc.vector.tensor_tensor(
                        ctx_sb[:, pr, t0 + h:t0 + TC], pctx[:, h:],
                        R_sb[:, h:], MULT)

                return [p_rcp, p_bcast, p_mult]

            with nc.named_scope("attn"):
                groups = [(tch, pr) for pr in range(2) for tch in range(NTC)]
                spill = []   # prev group leftovers: PV/rs pairs + tail
                for gi, (tch, pr) in enumerate(groups):
                    t0 = tch * TC
                    quota = sorted(extras.get(gi, []), key=lambda x: x[0])
                    qi = 0
                    pctx = ps_ctx.tile([P, TC], F32, tag="ps_ctx")
                    prs = ps_rs.tile([P, TC], F32, tag="ps_rs",
                                     name=f"prs_{gi}")
                    exps = [None] * (NST // 2)
                    for st in range(NST):
                        s0 = st * P
                        pss = ps_s.tile([P, 2 * TC], F32, tag="ps_s")
                        nc.tensor.matmul(
                            pss[:, :TC], KT_sb[0:64, pr, s0:s0 + P],
                            QT_sb[0:64, pr, t0:t0 + TC],
                            start=True, stop=True, tile_position=(0, 0))
                        nc.tensor.matmul(
                            pss[:, TC:], KT_sb[64:P, pr, s0:s0 + P],
                            QT_sb[64:P, pr, t0:t0 + TC],
                            start=True, stop=True, tile_position=(64, 0))
                        if st % 2 == 0:
                            exps[st // 2] = expool.tile(
                                [P, 2, 2, TC], F8, tag="expT",
                                name=f"ex_{gi}_{st // 2}")
                        nc.scalar.activation(
                            exps[st // 2][:, st % 2, :, :], pss[:],
                            mybir.ActivationFunctionType.Exp,
                            scale=EXP_SCALE)
                        # one extra-work quantum per iteration
                        while qi < len(quota) and quota[qi][0] <= st:
                            quota[qi][1]()
                            qi += 1
                            break
                        # one prev-group spill op per iteration
                        if st < len(spill):
                            spill[st]()
                        # this group's lagged PV / row-sum pairs
                        if st >= 4:
                            s = st - 4
                            pv(pctx, exps[s // 2], s, pr)
                        if st >= 3 and st % 2 == 1:
                            rs_pair(prs, exps[(st - 3) // 2], (st - 3) // 2)
                    while qi < len(quota):
                        quota[qi][1]()
                        qi += 1
                    spill = [
                        (lambda pc=pctx, e=exps[6], p=pr: pv(pc, e, 12, p)),
                        (lambda pc=pctx, e=exps[6], p=pr: pv(pc, e, 13, p)),
                        (lambda pc=pctx, e=exps[7], p=pr: pv(pc, e, 14, p)),
                        (lambda pc=pctx, e=exps[7], p=pr: pv(pc, e, 15, p)),
                        (lambda pr_=prs, e=exps[7]: rs_pair(pr_, e, 7)),
                    ] + tail_pieces(gi, tch, pr, pctx, prs)
                # last group's leftovers
                for fn in spill:
                    fn()

            with nc.named_scope("outproj"):
                for tt in range(12, NST):
                    for ec2 in range(2):
                        outproj_half(tt, ec2)
    nc.compile()
    return nc


_NC = None


def _get_nc():
    global _NC
    if _NC is None:
        _NC = build_kernel()
    return _NC


def make_in_maps(hidden_states, Wq, bq, Wk, bk, Wv, bv, Wo, bo, gate):
    f = np.float32
    f8 = ml_dtypes.float8_e4m3
    b16 = ml_dtypes.bfloat16
    hidden_states = np.asarray(hidden_states, f)
    Wq, bq = np.asarray(Wq, f), np.asarray(bq, f)
    Wk, bk = np.asarray(Wk, f), np.asarray(bk, f)
    Wv, bv = np.asarray(Wv, f), np.asarray(bv, f)
    Wo, bo = np.asarray(Wo, f), np.asarray(bo, f)
    gate = np.asarray(gate, f)

    hT_b = [np.ascontiguousarray(hidden_states[b].T)
            .reshape(NEC, P, T).astype(f8) for b in range(B)]
    on2_np = np.zeros((P, 2, 2, P), f8)
    on2_np[:, 0, :, 0] = 1.0   # head-A row-sum -> psum row 0
    on2_np[:, 1, :, 1] = 1.0   # head-B row-sum -> psum row 1
    sel2_np = np.zeros((2, P), b16)
    sel2_np[0, 0:64] = CC / AV
    sel2_np[1, 64:P] = CC / AV

    in_maps = []
    consts = []
    for core in range(NCORES):
        b, hg = divmod(core, 4)
        hs = [4 * hg + i for i in range(4)]

        def pack_qk(W):
            outw = np.empty((P, 2, 4, 2, P), f)
            for pr in range(2):
                pair = np.concatenate(
                    [W[hs[2 * pr]], W[hs[2 * pr + 1]]], axis=1)  # [E, 128]
                outw[:, pr] = (AQ * pair).reshape(4, 2, P, P).transpose(
                    2, 0, 1, 3)
            return outw.astype(f8)

        wv_np = np.concatenate([Wv[h] for h in hs], axis=1)  # [E, 256]
        wv_np = (AV * wv_np).reshape(4, 2, P, 256).transpose(
            2, 0, 1, 3).astype(f8)
        wo_np = np.empty((2, P, E), f)
        bq_np = np.empty((P, 2), f)
        bk_np = np.empty((P, 2), f)
        for pr in range(2):
            h0, h1 = hs[2 * pr], hs[2 * pr + 1]
            wo_np[pr] = np.concatenate(
                [gate[h0] * Wo[h0], gate[h1] * Wo[h1]], axis=0)  # [128, E]
            bq_np[:, pr] = AQ * np.concatenate([bq[h0], bq[h1]])
            bk_np[:, pr] = AQ * np.concatenate([bk[h0], bk[h1]])
        # constant term: sum_h gate_h * (bo_h + bv_h @ Wo_h)   [E]
        cst = sum(gate[h] * (bo[h] + bv[h] @ Wo[h]) for h in hs)
        consts.append(np.asarray(cst, f))
        in_maps.append(dict(
            hT=np.ascontiguousarray(hT_b[b]),
            wq=np.ascontiguousarray(pack_qk(Wq)),
            wk=np.ascontiguousarray(pack_qk(Wk)),
            wv=np.ascontiguousarray(wv_np),
            wo=np.ascontiguousarray(
                wo_np.transpose(1, 0, 2).astype(b16)),
            bq=bq_np, bk=bk_np,
            on2=on2_np, sel2=sel2_np,
        ))
    return in_maps, consts


def kernel(hidden_states, Wq, bq, Wk, bk, Wv, bv, Wo, bo, gate, _trace=False,
           **run_kwargs):
    nc = _get_nc()
    in_maps, consts = make_in_maps(
        hidden_states, Wq, bq, Wk, bk, Wv, bv, Wo, bo, gate)
    res = bass_utils.run_bass_kernel_spmd(
        nc, in_maps, core_ids=list(range(NCORES)), trace=_trace, **run_kwargs)
    outs = [np.asarray(r["out"], np.float32) for r in res.results]
    full = np.stack([
        outs[0] + outs[1] + outs[2] + outs[3]
        + (consts[0] + consts[1] + consts[2] + consts[3])[None, :],
        outs[4] + outs[5] + outs[6] + outs[7]
        + (consts[4] + consts[5] + consts[6] + consts[7])[None, :],
    ]).astype(np.float32)
    kernel.last_result = res
    return full


# revision 36
# speedup vs baseline: 1.0053x; 1.0053x over previous
"""Gated multi-head self-attention on 8 Trainium2 NeuronCores.

Sharding: batch (B=2) x head-groups (4 groups of 4 heads) -> 8 cores.
Each core computes, for its batch b and its 4 heads:
    partial_out[t, e] = sum_h gate[h] * softmax(Q_h K_h^T / 8) V_h Wo_h
The host sums the 4 head-group partials per batch, adds the constant
term sum_h gate_h*(bo_h + bv_h Wo_h) (bv/bo commute past the softmax
normalization), and stacks the two batches.

Design (ScalarE exp is the floor: 128 x [128,1024] exps ~= 135us/core):
  - fp8e4 DoubleRow matmuls (2 moving cols/cycle, 256-deep contraction)
    for the QKV projections, PV, row-sums, and output projection; the
    scores matmuls stay bf16 (their d=64 contraction can't pair-split
    without restructuring Q/K layouts).
  - exp writes fp8 s-tile-PAIR tiles [128,(head,kt,t)]; PV consumes a
    pair per DoubleRow matmul (contraction 256 = two s-tiles).
  - softmax denominators: fp8 ones-matmuls ride the same exp pairs,
    accumulating both heads' row-sums into psum rows 0/1 of a dedicated
    bank; reciprocal_approx_fast + sel2 matmul broadcasts 1/R.
    This removes the former DVE chain-add stream entirely.
  - host-folded prescales keep fp8 operands in e4m3 normal range:
    Wq,Wk,bq,bk x64 (exp scale 0.125/4096), Wv x32, Wo*gate x64,
    ctx evicted as 32*ctx (sel2 value 1 = 32/32), output eviction
    multiplies by 1/2048 = 1/(64*32).
  - warm-up matmuls + six per-ec-pair interleaved projections ride the
    (bandwidth-bound) hT DMA window before attention starts.
  - attention groups pr-outer; per iteration: scores pair (row-tiled,
    both heads concurrent), one exp, at most one extra-work quantum,
    one spill op from the previous group's tail, this group's lag-4
    PV pair (even iterations) and lag-3 row-sum pair (odd iterations)
    -- so the in-order engine queues never stall the exp stream.
"""

import os
import numpy as np
import ml_dtypes
from contextlib import ExitStack

import concourse.bass as bass
import concourse.tile as tile
from concourse import bacc, mybir
from concourse import bass_utils

E, H, D = 1024, 16, 64
B, T = 2, 2048
NCORES = 8
P = 128
TC = 512          # t-chunk (PSUM bank = 512 fp32)
NTC = T // TC     # 4 t-chunks
NST = T // P      # 16 s-tiles
NEC = E // P      # 8 e-chunks

F32 = mybir.dt.float32
BF16 = mybir.dt.bfloat16
F8 = mybir.dt.float8e4
ADD = mybir.AluOpType.add
MULT = mybir.AluOpType.mult
DR = mybir.MatmulPerfMode.DoubleRow

AQ = 64.0   # Wq/Wk/bq/bk prescale  (exp scale divides by AQ^2)
AV = 32.0   # Wv prescale
BO = 64.0   # gate*Wo prescale (fp8 range)
CC = 32.0   # ctx_sb = CC * ctx_true  (sel2 value = CC/AV)
EXP_SCALE = 0.125 / (AQ * AQ)
OUT_SCALE = 1.0 / (BO * CC)


def build_kernel():
    nc = bacc.Bacc("TRN2", target_bir_lowering=False, debug=False,
                   num_devices=NCORES)
    hT = nc.dram_tensor("hT", [NEC, P, T], F8, kind="ExternalInput").ap()
    wq = nc.dram_tensor("wq", [P, 2, 4, 2, P], F8, kind="ExternalInput").ap()
    wk = nc.dram_tensor("wk", [P, 2, 4, 2, P], F8, kind="ExternalInput").ap()
    wv = nc.dram_tensor("wv", [P, 4, 2, 256], F8, kind="ExternalInput").ap()
    wo = nc.dram_tensor("wo", [P, 2, E], F8, kind="ExternalInput").ap()
    bq = nc.dram_tensor("bq", [P, 2], F32, kind="ExternalInput").ap()
    bk = nc.dram_tensor("bk", [P, 2], F32, kind="ExternalInput").ap()
    on2 = nc.dram_tensor("on2", [P, 2, 2, P], F8, kind="ExternalInput").ap()
    sel2 = nc.dram_tensor("sel2", [2, P], BF16, kind="ExternalInput").ap()
    out = nc.dram_tensor("out", [T, E], BF16, kind="ExternalOutput").ap()

    sim_safe = os.environ.get("BASS_SIM_SAFE", "") != ""

    with tile.TileContext(nc) as tc:
        with ExitStack() as ctx:
            persist = ctx.enter_context(tc.tile_pool(name="persist", bufs=1))
            expool = ctx.enter_context(tc.tile_pool(name="expool", bufs=10))
            work = ctx.enter_context(tc.tile_pool(name="work", bufs=8))
            ps_s = ctx.enter_context(tc.tile_pool(name="ps_s", bufs=2, space="PSUM"))
            ps_ctx = ctx.enter_context(tc.tile_pool(name="ps_ctx", bufs=2, space="PSUM"))
            ps_misc = ctx.enter_context(tc.tile_pool(name="ps_misc", bufs=1, space="PSUM"))
            ps_rs = ctx.enter_context(tc.tile_pool(name="ps_rs", bufs=1, space="PSUM"))

            # ---- persistent SBUF tensors ----
            hT_sb = persist.tile([P, NEC, T], F8, tag="hT")
            wq_sb = persist.tile([P, 2, 4, 2, P], F8, tag="wq")
            wk_sb = persist.tile([P, 2, 4, 2, P], F8, tag="wk")
            wv_sb = persist.tile([P, 4, 2, 256], F8, tag="wv")
            wo_sb = persist.tile([P, 2, E], F8, tag="wo")
            bq_sb = persist.tile([P, 2], F32, tag="bq")
            bk_sb = persist.tile([P, 2], F32, tag="bk")
            on2_sb = persist.tile([P, 2, 2, P], F8, tag="on2")
            sel2_sb = persist.tile([2, P], BF16, tag="sel2")
            QT_sb = persist.tile([P, 2, T], BF16, tag="QT")
            KT_sb = persist.tile([P, 2, T], BF16, tag="KT")
            # V2: zero-padded dual-head stationary for DoubleRow PV.
            # [s-part, s-tile, pr, head-ktile, d-col]: kt0 cols 0:64 = V_A,
            # kt1 cols 64:128 = V_B, rest stays zero (gpsimd memset).
            V2_sb = persist.tile([P, NST, 2, 2, P], F8, tag="V2")
            ctx_sb = persist.tile([P, 2, T], F8, tag="ctx")

            with nc.named_scope("load"):
                # PE warm-up matmuls on uninitialized SBUF keep the tensor
                # engine busy during the input DMAs so the HAM clock gate
                # ramps toward 8/8 before the real work starts.  Skipped
                # under BASS_SIM_SAFE (CoreSim rejects uninitialized reads;
                # these ops are timing-only).
                if not sim_safe:
                    for w in range(8):
                        psw = ps_misc.tile([P, TC], F32, tag="ps_misc",
                                           name=f"warm{w}")
                        nc.tensor.matmul(psw[:], KT_sb[0:64, 0, 0:P],
                                         QT_sb[0:64, 0, 0:TC],
                                         start=True, stop=True)
                    for w in range(3):
                        vw = work.tile([2, 64], BF16, tag="vw",
                                       name=f"vwarm{w}")
                        nc.vector.tensor_copy(vw[:], QT_sb[0:2, 0, 0:64])
                # Spread DMA issues across the three idle sequencers; hT
                # first (the pre-projections ride its arrivals), late-use
                # weights (wv/wo) last.
                nc.scalar.dma_start(bq_sb[:], bq)
                nc.scalar.dma_start(bk_sb[:], bk)
                for ec in range(NEC):
                    for hp in range(2):
                        eng = (nc.sync, nc.gpsimd, nc.scalar)[(2 * ec + hp) % 3]
                        eng.dma_start(hT_sb[hp * 64:(hp + 1) * 64, ec, :],
                                      hT[ec][hp * 64:(hp + 1) * 64])
                nc.sync.dma_start(wk_sb[:], wk)
                nc.scalar.dma_start(wq_sb[:], wq)
                nc.gpsimd.dma_start(wv_sb[:], wv)
                nc.gpsimd.dma_start(wo_sb[:], wo)
                nc.sync.dma_start(on2_sb[:], on2)
                nc.sync.dma_start(sel2_sb[:], sel2)
                nc.gpsimd.memset(V2_sb[:], 0.0)
                # Pre-warm the ACT exp table set (~2.7us table load) during
                # the DMA window so the first real exp doesn't pay it.
                actw = work.tile([P, 2], F32, tag="actw", name="actw")
                nc.scalar.activation(
                    actw[:], bq_sb[:],
                    mybir.ActivationFunctionType.Exp, scale=0.0)

            def qk_matmuls(w_sb, dst, pr, tch):
                for ecp in range(4):
                    nc.tensor.matmul(
                        dst, w_sb[:, pr, ecp, :, :],
                        hT_sb[:, 2 * ecp:2 * ecp + 2,
                              tch * TC:(tch + 1) * TC],
                        start=(ecp == 0), stop=(ecp == 3),
                        perf_mode=DR, skip_group_check=True)

            def proj_qk(w_sb, b_sb, dst, pr, tch):
                """One [128, TC] chunk of the Q or K projection (+bias)."""
                ps = ps_misc.tile([P, TC], F32, tag="ps_misc",
                                  name=f"pqk_{pr}_{tch}")
                qk_matmuls(w_sb, ps[:], pr, tch)
                nc.vector.tensor_scalar(
                    dst[:, pr, tch * TC:(tch + 1) * TC], ps[:],
                    b_sb[:, pr:pr + 1], None, ADD)

            def proj_v(st):
                """V rows for s-tile st -> zero-padded V2 stationary."""
                ps = ps_misc.tile([P, TC], F32, tag="ps_misc")
                psv = ps[:, :256]
                for ecp in range(4):
                    nc.tensor.matmul(
                        psv, hT_sb[:, 2 * ecp:2 * ecp + 2,
                                   st * P:(st + 1) * P],
                        wv_sb[:, ecp, :, :],
                        start=(ecp == 0), stop=(ecp == 3),
                        perf_mode=DR, skip_group_check=True)
                for pr in range(2):
                    for h in range(2):
                        c = (2 * pr + h) * 64
                        nc.vector.tensor_copy(
                            V2_sb[:, st, pr, h, 64 * h:64 * h + 64],
                            psv[:, c:c + 64])

            def outproj_half(tt, ec2, pool=None):
                """Half the output projection for t-tile tt -> HBM.
                One DoubleRow matmul: pr pair is the k-tile dim (d=256)."""
                p = pool or ps_misc
                pso = p.tile([P, TC], F32,
                             tag="ps_s" if p is ps_s else "ps_misc",
                             name=f"pso_{tt}_{ec2}")
                nc.tensor.matmul(
                    pso[:], ctx_sb[:, :, tt * P:(tt + 1) * P],
                    wo_sb[:, :, ec2 * TC:(ec2 + 1) * TC],
                    start=True, stop=True,
                    perf_mode=DR, skip_group_check=True)
                o_sb = work.tile([P, TC], BF16, tag="o")
                nc.vector.tensor_scalar(
                    o_sb[:], pso[:], OUT_SCALE, None, MULT)
                nc.sync.dma_start(
                    out[tt * P:(tt + 1) * P, ec2 * TC:(ec2 + 1) * TC],
                    o_sb[:])

            def pv(pctx, expair, s, pr):
                """ctx(both heads) += V2[s]^T @ exp[s] in one DoubleRow
                matmul: head k-tiles against the zero-padded stationary."""
                nc.tensor.matmul(
                    pctx[:], V2_sb[:, s, pr, :, :],
                    expair[:, s % 2, :, :],
                    start=(s == 0), stop=(s == NST - 1),
                    perf_mode=DR, skip_group_check=True)

            def rs_pair(prs, expair, p):
                """Row-sums of an s-tile pair: head h lands in prs row h
                (ones column h of the on2 variant), rest accumulates 0."""
                for h in range(2):
                    nc.tensor.matmul(
                        prs[:], on2_sb[:, h, :, :],
                        expair[:, :, h, :],
                        start=(p == 0 and h == 0),
                        stop=(p == NST // 2 - 1 and h == 1),
                        perf_mode=DR, skip_group_check=True)

            with nc.named_scope("qkv"):
                # Six projection chunks (K t0-t3, Q t0-t1) interleaved per
                # e-chunk-pair so the matmuls ride the hT DMA arrivals and
                # the whole K(pr0) set is done by the end of the load.
                psk0 = ps_misc.tile([P, TC], F32, tag="ps_misc", name="psk0")
                psq0 = ps_rs.tile([P, TC], F32, tag="ps_rs", name="psq0")
                pre_a = ps_s.tile([P, 2 * TC], F32, tag="ps_s", name="pre_a")
                pre_b = ps_s.tile([P, 2 * TC], F32, tag="ps_s", name="pre_b")
                jobs = [
                    (wk_sb, psk0[:], 0),            # K t0
                    (wq_sb, psq0[:], 0),            # Q t0
                    (wk_sb, pre_a[:, :TC], 1),      # K t1
                    (wk_sb, pre_a[:, TC:], 2),      # K t2
                    (wk_sb, pre_b[:, :TC], 3),      # K t3
                    (wq_sb, pre_b[:, TC:], 1),      # Q t1
                ]
                for ecp in range(4):
                    for w_sb, dst, t in jobs:
                        nc.tensor.matmul(
                            dst, w_sb[:, 0, ecp, :, :],
                            hT_sb[:, 2 * ecp:2 * ecp + 2,
                                  t * TC:(t + 1) * TC],
                            start=(ecp == 0), stop=(ecp == 3),
                            perf_mode=DR, skip_group_check=True)
                nc.vector.tensor_scalar(
                    KT_sb[:, 0, 0:TC], psk0[:], bk_sb[:, 0:1], None, ADD)
                nc.vector.tensor_scalar(
                    QT_sb[:, 0, 0:TC], psq0[:], bq_sb[:, 0:1], None, ADD)
                nc.vector.tensor_scalar(
                    KT_sb[:, 0, TC:2 * TC], pre_a[:, :TC],
                    bk_sb[:, 0:1], None, ADD)
                nc.vector.tensor_scalar(
                    KT_sb[:, 0, 2 * TC:3 * TC], pre_a[:, TC:],
                    bk_sb[:, 0:1], None, ADD)
                nc.vector.tensor_scalar(
                    KT_sb[:, 0, 3 * TC:4 * TC], pre_b[:, :TC],
                    bk_sb[:, 0:1], None, ADD)
                nc.vector.tensor_scalar(
                    QT_sb[:, 0, TC:2 * TC], pre_b[:, TC:],
                    bq_sb[:, 0:1], None, ADD)

            # Extra-work quanta interleaved one-per-iteration into the
            # attention groups as (min_iteration, fn).
            def QK(wb, bb, dstb, pr, t):
                return lambda: proj_qk(wb, bb, dstb, pr, t)

            extras = {
                0: [(0, QK(wq_sb, bq_sb, QT_sb, 0, 2)),
                    (1, QK(wq_sb, bq_sb, QT_sb, 0, 3))]
                   + [(2 + j, (lambda st=st: proj_v(st)))
                      for j, st in enumerate(range(0, 14))],
                1: [(j, (lambda st=st: proj_v(st)))
                    for j, st in enumerate(range(14, NST))]
                   + [(2 + j, QK(wk_sb, bk_sb, KT_sb, 1, t))
                      for j, t in enumerate(range(4))]
                   + [(6, QK(wq_sb, bq_sb, QT_sb, 1, 0))],
                2: [(0, QK(wq_sb, bq_sb, QT_sb, 1, 1)),
                    (1, QK(wq_sb, bq_sb, QT_sb, 1, 2)),
                    (2, QK(wq_sb, bq_sb, QT_sb, 1, 3))],
                5: [(8 + j, (lambda tt=tt, e=e: outproj_half(tt, e)))
                    for j, (tt, e) in enumerate(
                        (t, x) for t in range(0, 4) for x in range(2))],
                6: [(8 + j, (lambda tt=tt, e=e: outproj_half(tt, e)))
                    for j, (tt, e) in enumerate(
                        (t, x) for t in range(4, 8) for x in range(2))],
                7: [(8 + j, (lambda tt=tt, e=e: outproj_half(tt, e)))
                    for j, (tt, e) in enumerate(
                        (t, x) for t in range(8, 12) for x in range(2))],
            }

            # ---- attention: 8 groups, pr-outer ----
            def tail_pieces(gi, tch, pr, pctx, prs):
                t0 = tch * TC
                state = {}

                def p_rcp():
                    rcp_sb = work.tile([2, TC], F32, tag="rcp",
                                       name=f"rcp_{gi}")
                    nc.vector.reciprocal_approx_fast(rcp_sb[:], prs[0:2, :])
                    rcp_bf = work.tile([2, TC], BF16, tag="rcpb")
                    nc.vector.tensor_copy(rcp_bf[:], rcp_sb[:])
                    state["rcp_bf"] = rcp_bf

                def p_bcast():
                    pR = ps_misc.tile([P, TC], F32, tag="ps_misc",
                                      name=f"pR_{gi}")
                    nc.tensor.matmul(pR[:], sel2_sb[:], state["rcp_bf"][:],
                                     start=True, stop=True,
                                     skip_group_check=True)
                    state["pR"] = pR

                def p_mult():
                    R_sb = work.tile([P, TC], F32, tag="R", name=f"R_{gi}")
                    nc.vector.tensor_copy(R_sb[:], state["pR"][:])
                    h = TC // 2
                    nc.vector.tensor_tensor(
                        ctx_sb[:, pr, t0:t0 + h], pctx[:, :h],
                        R_sb[:, :h], MULT)
                    nc.vector.tensor_tensor(
                        ctx_sb[:, pr, t0 + h:t0 + TC], pctx[:, h:],
                        R_sb[:, h:], MULT)

                return [p_rcp, p_bcast, p_mult]

            with nc.named_scope("attn"):
                groups = [(tch, pr) for pr in range(2) for tch in range(NTC)]
                spill = []   # prev group leftovers: PV/rs pairs + tail
                for gi, (tch, pr) in enumerate(groups):
                    t0 = tch * TC
                    last = gi == len(groups) - 1
                    quota = sorted(extras.get(gi, []), key=lambda x: x[0])
                    qi = 0
                    pctx = ps_ctx.tile([P, TC], F32, tag="ps_ctx")
                    prs = ps_rs.tile([P, TC], F32, tag="ps_rs",
                                     name=f"prs_{gi}")
                    exps = [None] * (NST // 2)
                    for st in range(NST):
                        s0 = st * P
                        # independent work first: the scores matmul below
                        # may stall on its psum ring slot (freed by the
                        # lag-2 exp), so everything else runs ahead of it.
                        while qi < len(quota) and quota[qi][0] <= st:
                            quota[qi][1]()
                            qi += 1
                            break
                        if st < len(spill):
                            spill[st]()
                        # this group's lagged PV / row-sum pairs
                        if st >= 4:
                            pv(pctx, exps[(st - 4) // 2], st - 4, pr)
                            if last and st == 14:
                                # catch-up so the final tail is short
                                pv(pctx, exps[6], 12, pr)
                            if last and st == 15:
                                pv(pctx, exps[6], 13, pr)
                                pv(pctx, exps[7], 14, pr)
                        if st >= 3 and st % 2 == 1:
                            rs_pair(prs, exps[(st - 3) // 2], (st - 3) // 2)
                        pss = ps_s.tile([P, 2 * TC], F32, tag="ps_s")
                        nc.tensor.matmul(
                            pss[:, :TC], KT_sb[0:64, pr, s0:s0 + P],
                            QT_sb[0:64, pr, t0:t0 + TC],
                            start=True, stop=True, tile_position=(0, 0))
                        nc.tensor.matmul(
                            pss[:, TC:], KT_sb[64:P, pr, s0:s0 + P],
                            QT_sb[64:P, pr, t0:t0 + TC],
                            start=True, stop=True, tile_position=(64, 0))
                        if st % 2 == 0:
                            exps[st // 2] = expool.tile(
                                [P, 2, 2, TC], F8, tag="expT",
                                name=f"ex_{gi}_{st // 2}")
                        nc.scalar.activation(
                            exps[st // 2][:, st % 2, :, :], pss[:],
                            mybir.ActivationFunctionType.Exp,
                            scale=EXP_SCALE)
                    while qi < len(quota):
                        quota[qi][1]()
                        qi += 1
                    if last:
                        spill = [
                            (lambda pc=pctx, e=exps[7], p=pr: pv(pc, e, 15, p)),
                            (lambda pr_=prs, e=exps[7]: rs_pair(pr_, e, 7)),
                        ] + tail_pieces(gi, tch, pr, pctx, prs)
                    else:
                        spill = [
                            (lambda pc=pctx, e=exps[6], p=pr: pv(pc, e, 12, p)),
                            (lambda pc=pctx, e=exps[6], p=pr: pv(pc, e, 13, p)),
                            (lambda pc=pctx, e=exps[7], p=pr: pv(pc, e, 14, p)),
                            (lambda pc=pctx, e=exps[7], p=pr: pv(pc, e, 15, p)),
                            (lambda pr_=prs, e=exps[7]: rs_pair(pr_, e, 7)),
                        ] + tail_pieces(gi, tch, pr, pctx, prs)
                # last group's leftovers
                for fn in spill:
                    fn()

            with nc.named_scope("outproj"):
                # attention is done: ping-pong the freed score-psum slots
                # so consecutive halves overlap (MM of i+1 vs evict of i)
                for j, (tt, ec2) in enumerate(
                        (t, e) for t in range(12, NST) for e in range(2)):
                    outproj_half(tt, ec2, pool=(ps_s, ps_misc)[j % 2])
    nc.compile()
    return nc


_NC = None


def _get_nc():
    global _NC
    if _NC is None:
        _NC = build_kernel()
    return _NC


def make_in_maps(hidden_states, Wq, bq, Wk, bk, Wv, bv, Wo, bo, gate):
    f = np.float32
    f8 = ml_dtypes.float8_e4m3
    b16 = ml_dtypes.bfloat16
    hidden_states = np.asarray(hidden_states, f)
    Wq, bq = np.asarray(Wq, f), np.asarray(bq, f)
    Wk, bk = np.asarray(Wk, f), np.asarray(bk, f)
    Wv, bv = np.asarray(Wv, f), np.asarray(bv, f)
    Wo, bo = np.asarray(Wo, f), np.asarray(bo, f)
    gate = np.asarray(gate, f)

    hT_b = [np.ascontiguousarray(hidden_states[b].T)
            .reshape(NEC, P, T).astype(f8) for b in range(B)]
    on2_np = np.zeros((P, 2, 2, P), f8)
    on2_np[:, 0, :, 0] = 1.0   # head-A row-sum -> psum row 0
    on2_np[:, 1, :, 1] = 1.0   # head-B row-sum -> psum row 1
    sel2_np = np.zeros((2, P), b16)
    sel2_np[0, 0:64] = CC / AV
    sel2_np[1, 64:P] = CC / AV

    in_maps = []
    consts = []
    for core in range(NCORES):
        b, hg = divmod(core, 4)
        hs = [4 * hg + i for i in range(4)]

        def pack_qk(W):
            outw = np.empty((P, 2, 4, 2, P), f)
            for pr in range(2):
                pair = np.concatenate(
                    [W[hs[2 * pr]], W[hs[2 * pr + 1]]], axis=1)  # [E, 128]
                outw[:, pr] = (AQ * pair).reshape(4, 2, P, P).transpose(
                    2, 0, 1, 3)
            return outw.astype(f8)

        wv_np = np.concatenate([Wv[h] for h in hs], axis=1)  # [E, 256]
        wv_np = (AV * wv_np).reshape(4, 2, P, 256).transpose(
            2, 0, 1, 3).astype(f8)
        wo_np = np.empty((2, P, E), f)
        bq_np = np.empty((P, 2), f)
        bk_np = np.empty((P, 2), f)
        for pr in range(2):
            h0, h1 = hs[2 * pr], hs[2 * pr + 1]
            wo_np[pr] = BO * np.concatenate(
                [gate[h0] * Wo[h0], gate[h1] * Wo[h1]], axis=0)  # [128, E]
            bq_np[:, pr] = AQ * np.concatenate([bq[h0], bq[h1]])
            bk_np[:, pr] = AQ * np.concatenate([bk[h0], bk[h1]])
        # constant term: sum_h gate_h * (bo_h + bv_h @ Wo_h)   [E]
        cst = sum(gate[h] * (bo[h] + bv[h] @ Wo[h]) for h in hs)
        consts.append(np.asarray(cst, f))
        in_maps.append(dict(
            hT=np.ascontiguousarray(hT_b[b]),
            wq=np.ascontiguousarray(pack_qk(Wq)),
            wk=np.ascontiguousarray(pack_qk(Wk)),
            wv=np.ascontiguousarray(wv_np),
            wo=np.ascontiguousarray(
                wo_np.transpose(1, 0, 2).astype(f8)),
            bq=bq_np, bk=bk_np,
            on2=on2_np, sel2=sel2_np,
        ))
    return in_maps, consts


def kernel(hidden_states, Wq, bq, Wk, bk, Wv, bv, Wo, bo, gate, _trace=False,
           **run_kwargs):
    nc = _get_nc()
    in_maps, consts = make_in_maps(
        hidden_states, Wq, bq, Wk, bk, Wv, bv, Wo, bo, gate)
    res = bass_utils.run_bass_kernel_spmd(
        nc, in_maps, core_ids=list(range(NCORES)), trace=_trace, **run_kwargs)
    outs = [np.asarray(r["out"], np.float32) for r in res.results]
    full = np.stack([
        outs[0] + outs[1] + outs[2] + outs[3]
        + (consts[0] + consts[1] + consts[2] + consts[3])[None, :],
        outs[4] + outs[5] + outs[6] + outs[7]
        + (consts[4] + consts[5] + consts[6] + consts[7])[None, :],
    ]).astype(np.float32)
    kernel.last_result = res
    return full
